# revision 11
# baseline (speedup 1.0000x reference)
"""Branched feed-forward (4-phase MoE-style FF) on 8 Trainium2 NeuronCores.

Reference computation (B=32, S=1024, D=1024, P=4, F=4096):
    xs = x.reshape(B, P, S//P, D)              # static contiguous phase split
    h  = relu(xs @ W1[p] + b1[p])              # per-phase FF, D -> F
    y  = h @ W2[p] + b2[p]                     # F -> D
    out = y.reshape(B, S, D)

Sharding: 8 cores = 4 phases x 2 F-halves (expert parallel + FF-width
parallel).  Core c handles phase p = c//2, F-half fh = c%2: it computes a
partial y (contraction over its half of F) for ALL 8192 tokens of its
phase.  Host sums the two partials per phase and adds b2 (cheap numpy).

Per-core kernel (all weights SBUF-resident, fp32):
    for each token block (TT=256 tokens, 32 blocks):
        FF1: h[ft, :] = relu( sum_dc W1[dc,ft].T @ xT[dc, :] + b1[ft] )
        FF2: y[dt, :] = sum_fc W2[fc,dt].T @ h[fc, :]
Matmuls run as float32r (full-rate fp32 PE mode, one col/cycle).
"""

import numpy as np

import concourse.bass as bass
import concourse.mybir as mybir
import concourse.tile as tile
from concourse.bass import ts
from concourse.bass_utils import run_bass_kernel_spmd

# Problem dims (hardcoded per contest contract)
B, S, D = 32, 1024, 1024
P, F = 4, 4096
N_CORES = 8

# Per-core dims
FH = F // 2          # F half per core = 2048
T = B * (S // P)     # tokens per phase = 8192
TT = 256             # token block (matmul moving free dim)
TB = T // TT         # 32 token blocks
DC = D // 128        # 8 contraction chunks for FF1 / out tiles for FF2
FT = FH // 128       # 16 out tiles for FF1 / contraction chunks for FF2

F32 = mybir.dt.float32
F32R = mybir.dt.float32r


def _split_excess_waits(nc, cap=1):
    """The pinned walrus build rejects instructions carrying more than one
    sync wait ("Too many sync wait commands").  Hoist excess waits onto
    same-engine NoOps inserted immediately before the instruction — waiting
    A then B sequentially is equivalent to waiting A AND B atomically."""
    n = 0
    for fn in nc.m.functions:
        for blk in fn.blocks:
            out = []
            changed = False
            for inst in blk.instructions:
                si = inst.sync_info
                waits = list(si.on_wait) if si is not None else []
                if len(waits) > cap:
                    changed = True
                    for w in waits[:-cap]:
                        n += 1
                        out.append(
                            mybir.InstNoOp(
                                name=f"wsplit-{n}",
                                engine=inst.engine,
                                ins=[],
                                outs=[],
                                sync_info=mybir.SyncInfo(
                                    on_wait=[w], on_update=[]
                                ),
                            )
                        )
                    si.on_wait = waits[-cap:]
                out.append(inst)
            if changed:
                blk.instructions = out


def build_bass(reps=1):
    """Build the per-core Bass program.  `reps` repeats the whole compute
    loop inside one NEFF (same data, same outputs) — used by test.py to
    measure pure on-device time via the slope between rep counts."""
    nc = bass.Bass()

    x_d = nc.dram_tensor("x", [TB, DC, 128, TT], F32R, kind="ExternalInput")
    w1_d = nc.dram_tensor("w1", [D, FH], F32R, kind="ExternalInput")
    w2_d = nc.dram_tensor("w2", [FH, D], F32R, kind="ExternalInput")
    b1_d = nc.dram_tensor("b1", [128, FT], F32, kind="ExternalInput")
    y_d = nc.dram_tensor("y", [TB, DC, 128, TT], F32, kind="ExternalOutput")

    with tile.TileContext(nc) as tc:
        with (
            tc.tile_pool(name="weights", bufs=1) as wpool,
            tc.tile_pool(name="xin", bufs=3) as xpool,
            tc.tile_pool(name="hbuf", bufs=2) as hpool,
            tc.tile_pool(name="yout", bufs=4) as ypool,
            tc.tile_pool(name="psum", bufs=8, space="PSUM") as psum,
        ):
            w1_s = wpool.tile([128, DC, FH], F32R)
            nc.sync.dma_start(w1_s[:], w1_d.rearrange("(dc p) f -> p dc f", p=128))
            w2_s = wpool.tile([128, FT, D], F32R)
            nc.sync.dma_start(w2_s[:], w2_d.rearrange("(fc p) d -> p fc d", p=128))
            b1_s = wpool.tile([128, FT], F32)
            nc.sync.dma_start(b1_s[:], b1_d[:])

            for tb in [t for _ in range(reps) for t in range(TB)]:
                x_t = xpool.tile([128, DC, TT], F32R, tag="x")
                nc.sync.dma_start(x_t[:], x_d[tb].rearrange("dc p t -> p dc t"))

                h_t = hpool.tile([128, FT, TT], F32R, tag="h")
                for ft in range(FT):
                    ps = psum.tile([128, TT], F32, tag="ps")
                    for dc in range(DC):
                        nc.tensor.matmul(
                            ps[:],
                            w1_s[:, dc, ts(ft, 128)],
                            x_t[:, dc, :],
                            start=(dc == 0),
                            stop=(dc == DC - 1),
                        )
                    nc.scalar.activation(
                        h_t[:, ft, :],
                        ps[:],
                        mybir.ActivationFunctionType.Relu,
                        bias=b1_s[:, ft : ft + 1],
                    )

                for dt_ in range(DC):
                    ps = psum.tile([128, TT], F32, tag="ps")
                    for fc in range(FT):
                        nc.tensor.matmul(
                            ps[:],
                            w2_s[:, fc, ts(dt_, 128)],
                            h_t[:, fc, :],
                            start=(fc == 0),
                            stop=(fc == FT - 1),
                        )
                    y_t = ypool.tile([128, TT], F32, tag="y")
                    nc.vector.tensor_copy(y_t[:], ps[:])
                    nc.sync.dma_start(y_d[tb, dt_], y_t[:])

    _split_excess_waits(nc)
    return nc


_CACHED_NC = None


def _get_nc():
    global _CACHED_NC
    if _CACHED_NC is None:
        _CACHED_NC = build_bass()
    return _CACHED_NC


def _shard_inputs(x, W1, b1, W2):
    """Build the 8 per-core input maps. Core c: phase c//2, F-half c%2."""
    in_maps = []
    for c in range(N_CORES):
        p, fh = divmod(c, 2)
        xs = np.ascontiguousarray(x.reshape(B, P, S // P, D)[:, p])  # [B,S/P,D]
        xt = xs.reshape(TB, TT, DC, 128).transpose(0, 2, 3, 1)  # [TB,DC,128,TT]
        w1 = np.ascontiguousarray(W1[p][:, fh * FH : (fh + 1) * FH])
        w2 = np.ascontiguousarray(W2[p][fh * FH : (fh + 1) * FH, :])
        b1c = np.ascontiguousarray(b1[p][fh * FH : (fh + 1) * FH].reshape(FT, 128).T)
        in_maps.append(
            {
                "x": np.ascontiguousarray(xt).astype(np.float32),
                "w1": w1.astype(np.float32),
                "w2": w2.astype(np.float32),
                "b1": b1c.astype(np.float32),
            }
        )
    return in_maps


def _unshard_outputs(results, b2):
    """results: list of 8 dicts with 'y' [TB,DC,128,TT] partial sums."""
    y = np.empty((B, P, S // P, D), dtype=np.float32)
    for p in range(P):
        ya = results[2 * p]["y"]
        yb = results[2 * p + 1]["y"]
        # [TB,DC,128,TT] -> [TB,TT,DC,128] -> [T, D]
        yp = (ya + yb).transpose(0, 3, 1, 2).reshape(T, D) + b2[p][None, :]
        y[:, p] = yp.reshape(B, S // P, D)
    return y.reshape(B, S, D)


def kernel(x, W1, b1, W2, b2, phases):
    """Full-input entry point. `phases` is unused: the reference's phase
    assignment is the static contiguous partition of the sequence."""
    x = np.asarray(x, dtype=np.float32)
    W1 = np.asarray(W1, dtype=np.float32)
    b1 = np.asarray(b1, dtype=np.float32)
    W2 = np.asarray(W2, dtype=np.float32)
    b2 = np.asarray(b2, dtype=np.float32)

    nc = _get_nc()
    in_maps = _shard_inputs(x, W1, b1, W2)
    res = run_bass_kernel_spmd(nc, in_maps, core_ids=list(range(N_CORES)))
    return _unshard_outputs(res.results, b2)


if __name__ == "__main__":
    rng = np.random.default_rng(0)
    x = rng.standard_normal((B, S, D), dtype=np.float32)
    W1 = (rng.random((P, D, F), dtype=np.float32) - 0.5) / np.sqrt(D)
    b1 = (rng.random((P, F), dtype=np.float32) - 0.5) / np.sqrt(D)
    W2 = (rng.random((P, F, D), dtype=np.float32) - 0.5) / np.sqrt(F)
    b2 = (rng.random((P, D), dtype=np.float32) - 0.5) / np.sqrt(F)
    phases = rng.integers(0, P, size=(B, S)).astype(np.int32)

    y = kernel(x, W1, b1, W2, b2, phases)

    xs = x.reshape(B, P, S // P, D)
    h = np.maximum(np.einsum("bpsd,pdf->bpsf", xs, W1) + b1[None, :, None, :], 0.0)
    yref = (np.einsum("bpsf,pfd->bpsd", h, W2) + b2[None, :, None, :]).reshape(B, S, D)
    err = np.linalg.norm(y - yref) / np.linalg.norm(yref)
    print("rel err:", err)
